# revision 12
# baseline (speedup 1.0000x reference)
"""Chamfer distance kernel for Trainium2 (8 NeuronCores, SPMD).

Problem: points_src/points_trg [16, 4096, 3] f32.
  D[b,i,j] = ||x_i||^2 + ||y_j||^2 - 2 x_i.y_j
  returns (min_i D, min_j D)  — two [16, 4096] f32 arrays.

Strategy:
  - Data-parallel over batch: 2 batches per core.
  - The distance matrix tile [128 i, 512 j] is produced by ONE K=13 fp32r
    matmul: the contraction dim carries an augmented vector
      a = [x1_c, x1_c, x2_c (c=0..2), s1, s2, 1, 1]
      b = [t1_c, t2_c, t1_c (c=0..2), 1, 1, q1, q2]
    where x = x1+x2 and t = -2y = t1+t2 are 2-term splits on the fp32r
    grid (11 explicit mantissa bits, measured on HW), s = ||x||^2,
    q = ||y||^2 split likewise.  Sum_k a_k b_k = D up to ~1e-6 abs.
  - PE streams 512-wide fp32r matmuls at 1 cycle/row into PSUM (fp32).
  - ACT converts each PSUM tile to fp16 in SBUF (exact RN, verified).
  - DVE computes row-min (free-dim halving TT-min tree + reduce) and
    col-min (elementwise TT-min accumulator over i-tiles) in fp16 2x mode.
  - Col accumulator partition-reduce via PE transpose + DVE 3D reduce.
  - Output [128,32] blocks are DVE stream-transposed for contiguous DMA.
"""

import sys

import numpy as np

for _p in ("/opt/trn_rl_repo",):
    if _p not in sys.path:
        sys.path.insert(0, _p)

import concourse.bass as bass
import concourse.tile as tile
from concourse import mybir
from concourse.bass_utils import run_bass_kernel_spmd

F32 = mybir.dt.float32
F32R = mybir.dt.float32r
F16 = mybir.dt.float16
MIN = mybir.AluOpType.min

B, N, C = 16, 4096, 3
NCORES = 8
BPC = B // NCORES          # batches per core
K = 13                     # augmented contraction length
NIT = N // 128             # i-tiles per batch (32)
NJC = N // 1024            # 1024-wide j-chunks per row (4)

_MAX_WAITS = 1             # this walrus build allows 1 sync wait / instruction
_DMA = "gpsimd"            # DMA issue engine: "gpsimd" (SWDGE) or "sync" (HWDGE)
_F_WARM = True             # ACT-table warmup block
_F_CHUNK_DMA = True        # chunked input loads
_F_CHUNK_COLFOLD = True    # chunked last-i-tile colfold w/ interleaved reduce



def _split_excess_waits(nc):
    """Move excess sync waits onto same-engine NOPs placed just before."""
    for bb in nc.main_func.blocks:
        il = bb.instructions
        i = 0
        while i < len(il):
            inst = il[i]
            si = inst.sync_info
            if si is not None and si.on_wait and len(si.on_wait) > _MAX_WAITS:
                waits = list(si.on_wait)
                extra, keep = waits[:-_MAX_WAITS], waits[-_MAX_WAITS:]
                nops = []
                for k in range(0, len(extra), _MAX_WAITS):
                    chunk = extra[k:k + _MAX_WAITS]
                    nop = mybir.InstNoOp(
                        name=f"{inst.name}-wsplit{k}",
                        engine=inst.engine,
                        bass_nofuse=True,
                        sync_info=mybir.SyncInfo(on_wait=chunk, on_update=[]),
                    )
                    nc.register_instruction(nop, overwrite=True)
                    nops.append(nop)
                inst.sync_info = mybir.SyncInfo(
                    on_wait=keep, on_update=list(si.on_update))
                for j, nop in enumerate(nops):
                    il.insert(i + j, nop)
                i += len(nops)
            i += 1


def _round11(x):
    """Round to the fp32r grid: 11 explicit mantissa bits, RN."""
    x = np.asarray(x, np.float64)
    m, e = np.frexp(x)
    step = np.ldexp(1.0, e - 12)
    with np.errstate(invalid="ignore"):
        r = np.round(x / np.where(step == 0, 1.0, step)) * step
    return np.where(x == 0.0, 0.0, r)


def _build_aug(x, y):
    """Host-side augmented operands.  x,y: [B, N, 3] f32.

    Returns A, Bm: [B, K, N] f32 with all entries on the fp32r grid.
    """
    x = np.asarray(x, np.float64)
    y = np.asarray(y, np.float64)
    A = np.zeros((B, K, N), np.float64)
    Bm = np.zeros((B, K, N), np.float64)

    x1 = _round11(x)
    x2 = _round11(x - x1)
    t = -2.0 * y
    t1 = _round11(t)
    t2 = _round11(t - t1)
    for c in range(C):
        A[:, 3 * c + 0] = x1[:, :, c]
        A[:, 3 * c + 1] = x1[:, :, c]
        A[:, 3 * c + 2] = x2[:, :, c]
        Bm[:, 3 * c + 0] = t1[:, :, c]
        Bm[:, 3 * c + 1] = t2[:, :, c]
        Bm[:, 3 * c + 2] = t1[:, :, c]

    s = np.sum(x * x, axis=-1)
    s1 = _round11(s)
    s2 = _round11(s - s1)
    q = np.sum(y * y, axis=-1)
    q1 = _round11(q)
    q2 = _round11(q - q1)
    A[:, 9] = s1
    A[:, 10] = s2
    A[:, 11] = 1.0
    A[:, 12] = 1.0
    Bm[:, 9] = 1.0
    Bm[:, 10] = 1.0
    Bm[:, 11] = q1
    Bm[:, 12] = q2
    return A.astype(np.float32), Bm.astype(np.float32)


def _trace():
    """Build the SPMD per-core program.  Each core: BPC batches."""
    nc = bass.Bass()
    a_in = nc.declare_dram_parameter("a", [BPC, K, N], F32R, isOutput=False)
    b_in = nc.declare_dram_parameter("bm", [BPC, K, N], F32R, isOutput=False)
    id_in = nc.declare_dram_parameter("ident", [128, 128], F16, isOutput=False)
    omin1 = nc.declare_dram_parameter("omin1", [BPC, N], F32, isOutput=True)
    omin2 = nc.declare_dram_parameter("omin2", [BPC, N], F32, isOutput=True)

    with tile.TileContext(nc) as tc:
        with (
            tc.tile_pool(name="inp", bufs=1) as inp,
            tc.tile_pool(name="work", bufs=2) as work,
            tc.tile_pool(name="mm", bufs=3, space="PSUM") as mmp,
            tc.tile_pool(name="tps", bufs=2, space="PSUM") as tps,
        ):
            ident = inp.tile([128, 128], F16, tag="ident")
            getattr(nc, _DMA).dma_start(out=ident[:], in_=id_in[:])
            # warm the ACT function table while the input DMAs run: a
            # normal-shaped copy of the (early, small) identity load
            if _F_WARM:
                warm = inp.tile([128, 128], F16, tag="warm")
                nc.scalar.copy(warm[:], ident[:])
            # chunked loads so batch-0 compute starts after its first chunk
            NCH = 4
            CW = N // NCH
            ta, tb = [], []
            for b in range(BPC):
                t1 = inp.tile([K, N], F32R, tag=f"ta{b}")
                t2 = inp.tile([K, N], F32R, tag=f"tb{b}")
                ta.append(t1)
                tb.append(t2)
            if _F_CHUNK_DMA:
                for b in range(BPC):
                    for ch in range(NCH):
                        sl = slice(CW * ch, CW * (ch + 1))
                        if ch == 0:
                            getattr(nc, _DMA).dma_start(out=ta[b][:, sl], in_=a_in[b][:, sl])
                        getattr(nc, _DMA).dma_start(out=tb[b][:, sl], in_=b_in[b][:, sl])
                    for ch in range(1, NCH):
                        sl = slice(CW * ch, CW * (ch + 1))
                        getattr(nc, _DMA).dma_start(out=ta[b][:, sl], in_=a_in[b][:, sl])
            else:
                for b in range(BPC):
                    getattr(nc, _DMA).dma_start(out=ta[b][:], in_=a_in[b])
                    getattr(nc, _DMA).dma_start(out=tb[b][:], in_=b_in[b])

            for b in range(BPC):
                G = work.tile([128, N], F16, tag="G")
                rows = work.tile([128, NIT], F32, tag="rows")
                cols = work.tile([128, NIT], F32, tag="cols")
                # per-i-tile 128-wide row-fold results, reduced in one
                # batched 3D reduce at the end of the batch
                FC = work.tile([128, NIT, 128], F16, tag="FC")

                for it in range(NIT):
                    lhsT = ta[b][:, 128 * it:128 * (it + 1)]
                    S = work.tile([128, N], F16, tag="S")
                    for jc in range(NJC):
                        pm = mmp.tile([128, 1024], F32, tag="pm")
                        for h in range(2):
                            j0 = 1024 * jc + 512 * h
                            nc.tensor.matmul(
                                pm[:, 512 * h:512 * (h + 1)],
                                lhsT,
                                tb[b][:, j0:j0 + 512],
                                start=True, stop=True)
                        nc.scalar.copy(
                            S[:, 1024 * jc:1024 * (jc + 1)], pm[:])

                    # row-min: halving TT-min tree in fp16 (2x mode)
                    F2 = work.tile([128, 2048], F16, tag="F2")
                    nc.vector.tensor_tensor(F2[:], S[:, :2048], S[:, 2048:], MIN)
                    F1 = work.tile([128, 1024], F16, tag="F1")
                    nc.vector.tensor_tensor(F1[:], F2[:, :1024], F2[:, 1024:], MIN)
                    F0 = work.tile([128, 512], F16, tag="F0")
                    nc.vector.tensor_tensor(F0[:], F1[:, :512], F1[:, 512:], MIN)
                    FA = work.tile([128, 256], F16, tag="FA")
                    nc.vector.tensor_tensor(FA[:], F0[:, :256], F0[:, 256:], MIN)
                    nc.vector.tensor_tensor(
                        FC[:, it, :], FA[:, :128], FA[:, 128:], MIN)

                    # col-min accumulate over i-tiles.  Last i-tile goes in
                    # 512-chunks so the partition-reduce of each finished G
                    # chunk pipelines instead of waiting for the full row.
                    if it == 0:
                        nc.vector.tensor_tensor(G[:], S[:], S[:], MIN)
                    elif it < NIT - 1 or not _F_CHUNK_COLFOLD:
                        nc.vector.tensor_tensor(G[:], G[:], S[:], MIN)
                    else:
                        for jt in range(8):
                            sl = slice(512 * jt, 512 * (jt + 1))
                            nc.vector.tensor_tensor(
                                G[:, sl], G[:, sl], S[:, sl], MIN)
                            # col-min partition reduce for this chunk
                            pt = tps.tile([128, 4, 128], F16, tag="pt")
                            for k2 in range(4):
                                j0 = 512 * jt + 128 * k2
                                nc.tensor.transpose(
                                    pt[:, k2, :], G[:, j0:j0 + 128], ident[:])
                            nc.vector.tensor_reduce(
                                cols[:, 4 * jt:4 * (jt + 1)], pt[:],
                                axis=mybir.AxisListType.X, op=MIN)
                    # first-half row-min finish off the critical tail
                    if it == NIT // 2:
                        nc.vector.tensor_reduce(
                            rows[:, :NIT // 2], FC[:, :NIT // 2, :],
                            axis=mybir.AxisListType.X, op=MIN)

                # batched row-min finish for the second half
                nc.vector.tensor_reduce(
                    rows[:, NIT // 2:], FC[:, NIT // 2:, :],
                    axis=mybir.AxisListType.X, op=MIN)

                if not _F_CHUNK_COLFOLD:
                    for jt in range(8):
                        pt = tps.tile([128, 4, 128], F16, tag="pt")
                        for k2 in range(4):
                            j0 = 512 * jt + 128 * k2
                            nc.tensor.transpose(
                                pt[:, k2, :], G[:, j0:j0 + 128], ident[:])
                        nc.vector.tensor_reduce(
                            cols[:, 4 * jt:4 * (jt + 1)], pt[:],
                            axis=mybir.AxisListType.X, op=MIN)

                # outputs: [128, 32] where [p, q] = out[128*q + p]
                # stream-transpose 32x32 blocks then 4 contiguous DMAs
                for src, dst in ((cols, omin1), (rows, omin2)):
                    tr = work.tile([128, NIT], F32, tag="tr")
                    nc.vector.transpose(tr[:], src[:])
                    w = dst[b].rearrange("(c k) -> c k", k=128)
                    for g in range(4):
                        getattr(nc, _DMA).dma_start(
                            out=w[:, 32 * g:32 * (g + 1)],
                            in_=tr[32 * g:32 * (g + 1), :])

    _split_excess_waits(nc)
    return nc


_NC_CACHE = None


def _get_nc():
    global _NC_CACHE
    if _NC_CACHE is None:
        _NC_CACHE = _trace()
    return _NC_CACHE


def _run(points_src, points_trg, trace=False, trace_kwargs=None):
    x = np.asarray(points_src, np.float32)
    y = np.asarray(points_trg, np.float32)
    assert x.shape == (B, N, C) and y.shape == (B, N, C)
    A, Bm = _build_aug(x, y)
    ident = np.eye(128, dtype=np.float16)
    in_maps = [
        {"a": np.ascontiguousarray(A[BPC * i:BPC * (i + 1)]),
         "bm": np.ascontiguousarray(Bm[BPC * i:BPC * (i + 1)]),
         "ident": ident}
        for i in range(NCORES)
    ]
    res = run_bass_kernel_spmd(
        _get_nc(), in_maps, list(range(NCORES)), trace=trace,
        **(trace_kwargs or {}))
    min1 = np.concatenate(
        [res.results[i]["omin1"] for i in range(NCORES)], axis=0)
    min2 = np.concatenate(
        [res.results[i]["omin2"] for i in range(NCORES)], axis=0)
    return (min1, min2), res


def kernel(points_src, points_trg):
    (min1, min2), _ = _run(points_src, points_trg)
    return min1, min2


# revision 14
# speedup vs baseline: 1.0037x; 1.0037x over previous
"""Chamfer distance kernel for Trainium2 (8 NeuronCores, SPMD).

Problem: points_src/points_trg [16, 4096, 3] f32.
  D[b,i,j] = ||x_i||^2 + ||y_j||^2 - 2 x_i.y_j
  returns (min_i D, min_j D)  — two [16, 4096] f32 arrays.

Strategy:
  - Data-parallel over batch: 2 batches per core.
  - The distance matrix tile [128 i, 512 j] is produced by ONE K=13 fp32r
    matmul: the contraction dim carries an augmented vector
      a = [x1_c, x1_c, x2_c (c=0..2), s1, s2, 1, 1]
      b = [t1_c, t2_c, t1_c (c=0..2), 1, 1, q1, q2]
    where x = x1+x2 and t = -2y = t1+t2 are 2-term splits on the fp32r
    grid (11 explicit mantissa bits, measured on HW), s = ||x||^2,
    q = ||y||^2 split likewise.  Sum_k a_k b_k = D up to ~1e-6 abs.
  - PE streams 512-wide fp32r matmuls at 1 cycle/row into PSUM (fp32).
  - ACT converts each PSUM tile to fp16 in SBUF (exact RN, verified).
  - DVE computes row-min (free-dim halving TT-min tree, finished by a
    per-batch 3D reduce) and col-min (elementwise TT-min accumulator
    over i-tiles) in fp16 2x mode.  DVE is the bottleneck engine at
    ~93% busy; cost-model wall ~335 us/core.
  - Col accumulator partition-reduce via PE transpose + DVE 3D reduce.
  - Output [128,32] blocks are DVE stream-transposed for contiguous DMA.

Numerics: outputs match the fp32 reference to ~3e-4 relative to the
output scale (dominated by the fp16 rounding of the distance values;
the matmul itself contributes ~1e-6).
"""

import sys

import numpy as np

for _p in ("/opt/trn_rl_repo",):
    if _p not in sys.path:
        sys.path.insert(0, _p)

import concourse.bass as bass
import concourse.tile as tile
from concourse import mybir
from concourse.bass_utils import run_bass_kernel_spmd

F32 = mybir.dt.float32
F32R = mybir.dt.float32r
F16 = mybir.dt.float16
MIN = mybir.AluOpType.min

B, N, C = 16, 4096, 3
NCORES = 8
BPC = B // NCORES          # batches per core
K = 13                     # augmented contraction length
NIT = N // 128             # i-tiles per batch (32)
NJC = N // 1024            # 1024-wide j-chunks per row (4)

_MAX_WAITS = 1             # this walrus build allows 1 sync wait / instruction
_DMA = "gpsimd"            # DMA issue engine: "gpsimd" (SWDGE) or "sync" (HWDGE)
_F_WARM = True             # ACT-table warmup block
_F_CHUNK_DMA = True        # chunked input loads
_F_CHUNK_COLFOLD = True    # chunked last-i-tile colfold w/ interleaved reduce



def _split_excess_waits(nc):
    """Move excess sync waits onto same-engine NOPs placed just before."""
    for bb in nc.main_func.blocks:
        il = bb.instructions
        i = 0
        while i < len(il):
            inst = il[i]
            si = inst.sync_info
            if si is not None and si.on_wait and len(si.on_wait) > _MAX_WAITS:
                waits = list(si.on_wait)
                extra, keep = waits[:-_MAX_WAITS], waits[-_MAX_WAITS:]
                nops = []
                for k in range(0, len(extra), _MAX_WAITS):
                    chunk = extra[k:k + _MAX_WAITS]
                    nop = mybir.InstNoOp(
                        name=f"{inst.name}-wsplit{k}",
                        engine=inst.engine,
                        bass_nofuse=True,
                        sync_info=mybir.SyncInfo(on_wait=chunk, on_update=[]),
                    )
                    nc.register_instruction(nop, overwrite=True)
                    nops.append(nop)
                inst.sync_info = mybir.SyncInfo(
                    on_wait=keep, on_update=list(si.on_update))
                for j, nop in enumerate(nops):
                    il.insert(i + j, nop)
                i += len(nops)
            i += 1


def _round11(x):
    """Round to the fp32r grid: 11 explicit mantissa bits, RN."""
    x = np.asarray(x, np.float64)
    m, e = np.frexp(x)
    step = np.ldexp(1.0, e - 12)
    with np.errstate(invalid="ignore"):
        r = np.round(x / np.where(step == 0, 1.0, step)) * step
    return np.where(x == 0.0, 0.0, r)


def _build_aug(x, y):
    """Host-side augmented operands.  x,y: [B, N, 3] f32.

    Returns A, Bm: [B, K, N] f32 with all entries on the fp32r grid.
    """
    x = np.asarray(x, np.float64)
    y = np.asarray(y, np.float64)
    A = np.zeros((B, K, N), np.float64)
    Bm = np.zeros((B, K, N), np.float64)

    x1 = _round11(x)
    x2 = _round11(x - x1)
    t = -2.0 * y
    t1 = _round11(t)
    t2 = _round11(t - t1)
    for c in range(C):
        A[:, 3 * c + 0] = x1[:, :, c]
        A[:, 3 * c + 1] = x1[:, :, c]
        A[:, 3 * c + 2] = x2[:, :, c]
        Bm[:, 3 * c + 0] = t1[:, :, c]
        Bm[:, 3 * c + 1] = t2[:, :, c]
        Bm[:, 3 * c + 2] = t1[:, :, c]

    s = np.sum(x * x, axis=-1)
    s1 = _round11(s)
    s2 = _round11(s - s1)
    q = np.sum(y * y, axis=-1)
    q1 = _round11(q)
    q2 = _round11(q - q1)
    A[:, 9] = s1
    A[:, 10] = s2
    A[:, 11] = 1.0
    A[:, 12] = 1.0
    Bm[:, 9] = 1.0
    Bm[:, 10] = 1.0
    Bm[:, 11] = q1
    Bm[:, 12] = q2
    return A.astype(np.float32), Bm.astype(np.float32)


def _trace():
    """Build the SPMD per-core program.  Each core: BPC batches."""
    nc = bass.Bass()
    a_in = nc.declare_dram_parameter("a", [BPC, K, N], F32R, isOutput=False)
    b_in = nc.declare_dram_parameter("bm", [BPC, K, N], F32R, isOutput=False)
    id_in = nc.declare_dram_parameter("ident", [128, 128], F16, isOutput=False)
    omin1 = nc.declare_dram_parameter("omin1", [BPC, N], F32, isOutput=True)
    omin2 = nc.declare_dram_parameter("omin2", [BPC, N], F32, isOutput=True)

    with tile.TileContext(nc) as tc:
        with (
            tc.tile_pool(name="inp", bufs=1) as inp,
            tc.tile_pool(name="work", bufs=2) as work,
            tc.tile_pool(name="mm", bufs=3, space="PSUM") as mmp,
            tc.tile_pool(name="tps", bufs=2, space="PSUM") as tps,
        ):
            ident = inp.tile([128, 128], F16, tag="ident")
            getattr(nc, _DMA).dma_start(out=ident[:], in_=id_in[:])
            # warm the ACT function table while the input DMAs run: a
            # normal-shaped copy of the (early, small) identity load
            if _F_WARM:
                warm = inp.tile([128, 128], F16, tag="warm")
                nc.scalar.copy(warm[:], ident[:])
            # chunked loads so batch-0 compute starts after its first chunk
            NCH = 4
            CW = N // NCH
            ta, tb = [], []
            for b in range(BPC):
                t1 = inp.tile([K, N], F32R, tag=f"ta{b}")
                t2 = inp.tile([K, N], F32R, tag=f"tb{b}")
                ta.append(t1)
                tb.append(t2)
            if _F_CHUNK_DMA:
                for b in range(BPC):
                    for ch in range(NCH):
                        sl = slice(CW * ch, CW * (ch + 1))
                        if ch == 0:
                            getattr(nc, _DMA).dma_start(out=ta[b][:, sl], in_=a_in[b][:, sl])
                        getattr(nc, _DMA).dma_start(out=tb[b][:, sl], in_=b_in[b][:, sl])
                    for ch in range(1, NCH):
                        sl = slice(CW * ch, CW * (ch + 1))
                        getattr(nc, _DMA).dma_start(out=ta[b][:, sl], in_=a_in[b][:, sl])
            else:
                for b in range(BPC):
                    getattr(nc, _DMA).dma_start(out=ta[b][:], in_=a_in[b])
                    getattr(nc, _DMA).dma_start(out=tb[b][:], in_=b_in[b])

            for b in range(BPC):
                G = work.tile([128, N], F16, tag="G")
                rows = work.tile([128, NIT], F32, tag="rows")
                cols = work.tile([128, NIT], F32, tag="cols")
                # per-i-tile 128-wide row-fold results, reduced in one
                # batched 3D reduce at the end of the batch
                FC = work.tile([128, NIT, 128], F16, tag="FC")

                for it in range(NIT):
                    lhsT = ta[b][:, 128 * it:128 * (it + 1)]
                    S = work.tile([128, N], F16, tag="S")
                    for jc in range(NJC):
                        pm = mmp.tile([128, 1024], F32, tag="pm")
                        for h in range(2):
                            j0 = 1024 * jc + 512 * h
                            nc.tensor.matmul(
                                pm[:, 512 * h:512 * (h + 1)],
                                lhsT,
                                tb[b][:, j0:j0 + 512],
                                start=True, stop=True)
                        nc.scalar.copy(
                            S[:, 1024 * jc:1024 * (jc + 1)], pm[:])

                    # row-min: halving TT-min tree in fp16 (2x mode)
                    F2 = work.tile([128, 2048], F16, tag="F2")
                    nc.vector.tensor_tensor(F2[:], S[:, :2048], S[:, 2048:], MIN)
                    F1 = work.tile([128, 1024], F16, tag="F1")
                    nc.vector.tensor_tensor(F1[:], F2[:, :1024], F2[:, 1024:], MIN)
                    F0 = work.tile([128, 512], F16, tag="F0")
                    nc.vector.tensor_tensor(F0[:], F1[:, :512], F1[:, 512:], MIN)
                    FA = work.tile([128, 256], F16, tag="FA")
                    nc.vector.tensor_tensor(FA[:], F0[:, :256], F0[:, 256:], MIN)
                    nc.vector.tensor_tensor(
                        FC[:, it, :], FA[:, :128], FA[:, 128:], MIN)

                    # col-min accumulate over i-tiles.  Last i-tile goes in
                    # 512-chunks so the partition-reduce of each finished G
                    # chunk pipelines instead of waiting for the full row.
                    if it == 0:
                        nc.vector.tensor_copy(G[:], S[:])
                    elif it < NIT - 1 or not _F_CHUNK_COLFOLD:
                        nc.vector.tensor_tensor(G[:], G[:], S[:], MIN)
                    else:
                        for jt in range(8):
                            sl = slice(512 * jt, 512 * (jt + 1))
                            nc.vector.tensor_tensor(
                                G[:, sl], G[:, sl], S[:, sl], MIN)
                            # col-min partition reduce for this chunk
                            pt = tps.tile([128, 4, 128], F16, tag="pt")
                            for k2 in range(4):
                                j0 = 512 * jt + 128 * k2
                                nc.tensor.transpose(
                                    pt[:, k2, :], G[:, j0:j0 + 128], ident[:])
                            nc.vector.tensor_reduce(
                                cols[:, 4 * jt:4 * (jt + 1)], pt[:],
                                axis=mybir.AxisListType.X, op=MIN)
                    # first-half row-min finish off the critical tail
                    if it == NIT // 2:
                        nc.vector.tensor_reduce(
                            rows[:, :NIT // 2], FC[:, :NIT // 2, :],
                            axis=mybir.AxisListType.X, op=MIN)

                # batched row-min finish for the second half
                nc.vector.tensor_reduce(
                    rows[:, NIT // 2:], FC[:, NIT // 2:, :],
                    axis=mybir.AxisListType.X, op=MIN)

                if not _F_CHUNK_COLFOLD:
                    for jt in range(8):
                        pt = tps.tile([128, 4, 128], F16, tag="pt")
                        for k2 in range(4):
                            j0 = 512 * jt + 128 * k2
                            nc.tensor.transpose(
                                pt[:, k2, :], G[:, j0:j0 + 128], ident[:])
                        nc.vector.tensor_reduce(
                            cols[:, 4 * jt:4 * (jt + 1)], pt[:],
                            axis=mybir.AxisListType.X, op=MIN)

                # outputs: [128, 32] where [p, q] = out[128*q + p]
                # stream-transpose 32x32 blocks then 4 contiguous DMAs
                for src, dst in ((cols, omin1), (rows, omin2)):
                    tr = work.tile([128, NIT], F32, tag="tr")
                    nc.vector.transpose(tr[:], src[:])
                    w = dst[b].rearrange("(c k) -> c k", k=128)
                    for g in range(4):
                        getattr(nc, _DMA).dma_start(
                            out=w[:, 32 * g:32 * (g + 1)],
                            in_=tr[32 * g:32 * (g + 1), :])

    _split_excess_waits(nc)
    return nc


_NC_CACHE = None


def _get_nc():
    global _NC_CACHE
    if _NC_CACHE is None:
        _NC_CACHE = _trace()
    return _NC_CACHE


def _run(points_src, points_trg, trace=False, trace_kwargs=None):
    x = np.asarray(points_src, np.float32)
    y = np.asarray(points_trg, np.float32)
    assert x.shape == (B, N, C) and y.shape == (B, N, C)
    A, Bm = _build_aug(x, y)
    ident = np.eye(128, dtype=np.float16)
    in_maps = [
        {"a": np.ascontiguousarray(A[BPC * i:BPC * (i + 1)]),
         "bm": np.ascontiguousarray(Bm[BPC * i:BPC * (i + 1)]),
         "ident": ident}
        for i in range(NCORES)
    ]
    res = run_bass_kernel_spmd(
        _get_nc(), in_maps, list(range(NCORES)), trace=trace,
        **(trace_kwargs or {}))
    min1 = np.concatenate(
        [res.results[i]["omin1"] for i in range(NCORES)], axis=0)
    min2 = np.concatenate(
        [res.results[i]["omin2"] for i in range(NCORES)], axis=0)
    return (min1, min2), res


def kernel(points_src, points_trg):
    (min1, min2), _ = _run(points_src, points_trg)
    return min1, min2
